# revision 39
# baseline (speedup 1.0000x reference)
"""Trainium2 Bass kernel for a 2-layer Longformer-style encoder.

Model: B=2, S=2048, F=438, H=768, NH=12, HD=64, one-sided window w=32, L=2.

Sharding: 8 cores, data-parallel over (batch, sequence-quarter). Each core
computes 512 output tokens from a 640-token local window (64-token halo on
each side covers the 2-layer receptive field), so no collectives are needed.

Device algorithm per core (uniform SPMD, 640 local tokens):
  - x0 = srcT_pad.T @ W_embT + (pos_emb + b_emb)           [token-major f32]
  - per layer:
      xT   = transpose(x)  bf16                             [feature-major]
      qT   = W_qT'.T @ xT, scaled by HD^-0.5 on host        [feature-major]
      kTp  = W_kT.T @ xT at free-offset 32 in a 768-wide
             zero-padded buffer                             [feature-major]
      per query tile t (software-pipelined; V window t+2 is
      GEMMed just-in-time to keep the PE dense/warm):
        per head h: ST = banded score matmuls               [n-major scores]
        P = exp(ST) * mask01  (post-exp multiplicative mask; zero-padded
            keys keep the un-masked scores finite)
        den[h,q] += ones-matmuls over P (replicated per head row-block)
        ctxT[64h:+64, t] += V_sh.T @ P                      [feature-major]
        rb = exp(-ln(max(den,1e-6)))  (reciprocal via Ln/Exp so the ACT
            engine never leaves the natural_log_exp table set)
        ctxT *= rb; fc: F = ctxT.T @ W_fcT + residual
        LN1 (rstd = exp(-0.5 ln(var+eps))) -> x1, transpose -> x1T
      H1T = relu(W_1T.T @ x1T)                              [feature-major]
      F2 = H1T.T @ W_2T + x1; LN2 -> x2
  - out = x2[64:576]
"""

import numpy as np
import ml_dtypes

B, S, F_DIM, H, NH, HD, W_ONE, L = 2, 2048, 438, 768, 12, 64, 32, 2
NCORES = 8
CHUNK = 512          # output tokens per core
HALO = 64            # per side
T_LOC = CHUNK + 2 * HALO   # 640 local tokens
NT = T_LOC // 128          # 5 query tiles
KPAD = T_LOC + 128         # 768 padded key width (full 128-wide second score block)
SPAN = 192                 # keys per query tile (128 + 2*32)
FK = 512                   # padded embedding contraction (438 -> 512)
# Large enough that masked positions contribute ~e^-50 ~ 2e-22 (negligible),
# small enough that a fully-masked (pad) query row keeps a nonzero softmax
# denominator -> no inf/NaN anywhere (pad rows are discarded on output).
MASK_NEG = -50.0

bf16 = ml_dtypes.bfloat16


def _np(x):
    return np.asarray(x)


def host_prep(inputs):
    """Split full inputs into shared weight arrays + per-core arrays."""
    src_seq = _np(inputs["src_seq"]).astype(np.float32)
    src_pos = _np(inputs["src_pos"]).astype(np.int32)
    pos_table = _np(inputs["pos_table"]).astype(np.float32)

    shared = {}
    qscale = float(HD) ** -0.5

    W_emb = _np(inputs["W_emb"]).astype(np.float32)        # [H, F]
    WembT = np.zeros((FK, H), np.float32)
    WembT[:F_DIM] = W_emb.T
    shared["wembT"] = WembT.astype(bf16)

    for l in range(L):
        Wq = _np(inputs["Wq"])[l].astype(np.float32)
        Wk = _np(inputs["Wk"])[l].astype(np.float32)
        Wv = _np(inputs["Wv"])[l].astype(np.float32)
        Wfc = _np(inputs["Wfc"])[l].astype(np.float32)
        W1 = _np(inputs["W1"])[l].astype(np.float32)
        W2 = _np(inputs["W2"])[l].astype(np.float32)
        shared[f"wqT{l}"] = (Wq.T * qscale).astype(bf16)   # [H_in, H_out]
        shared[f"wkT{l}"] = Wk.T.astype(bf16)
        shared[f"wvT{l}"] = Wv.T.astype(bf16)
        shared[f"wfcT{l}"] = Wfc.T.astype(bf16)
        shared[f"w1T{l}"] = W1.T.astype(bf16)
        shared[f"w2T{l}"] = W2.T.astype(bf16)
        shared[f"bq{l}"] = (_np(inputs["bq"])[l].astype(np.float32) * qscale)
        shared[f"bk{l}"] = _np(inputs["bk"])[l].astype(np.float32)
        shared[f"bv{l}"] = _np(inputs["bv"])[l].astype(np.float32)
        shared[f"bfc{l}"] = _np(inputs["bfc"])[l].astype(np.float32)
        shared[f"b1{l}"] = _np(inputs["b1"])[l].astype(np.float32)
        shared[f"b2{l}"] = _np(inputs["b2"])[l].astype(np.float32)
        shared[f"ln1g{l}"] = _np(inputs["ln1_g"])[l].astype(np.float32)
        shared[f"ln1b{l}"] = _np(inputs["ln1_b"])[l].astype(np.float32)
        shared[f"ln2g{l}"] = _np(inputs["ln2_g"])[l].astype(np.float32)
        shared[f"ln2b{l}"] = _np(inputs["ln2_b"])[l].astype(np.float32)

    b_emb = _np(inputs["b_emb"]).astype(np.float32)

    per_core = []
    for c in range(NCORES):
        b, q = divmod(c, NCORES // B)
        gstart = q * CHUNK - HALO
        lo, hi = max(gstart, 0), min(gstart + T_LOC, S)

        src_halo = np.zeros((T_LOC, F_DIM), np.float32)
        src_halo[lo - gstart: hi - gstart] = src_seq[b, lo:hi]
        srcT = np.zeros((FK, T_LOC), np.float32)
        srcT[:F_DIM] = src_halo.T

        pos_emb = np.zeros((T_LOC, H), np.float32)
        pos_emb[lo - gstart: hi - gstart] = pos_table[src_pos[b, lo:hi]]
        pos_emb += b_emb[None, :]

        # n-major multiplicative 0/1 mask per query tile, applied to
        # P = exp(scores) AFTER the exp: maskM[n, t, 0:128] covers span keys
        # 0..127, maskM[0:64, t, 128:256] covers span keys 128..191 (rows
        # 64:128 of the second block are always 0 - those score rows are
        # computed but unused). local key = 128t - 32 + n, query = 128t + q.
        maskM = np.zeros((128, NT, 512), np.float32)
        for t in range(NT):
            n = np.arange(SPAN)[:, None]
            qq = np.arange(128)[None, :]
            kl = 128 * t - 32 + n
            kg = gstart + kl
            band = np.abs(kl - (128 * t + qq)) <= W_ONE
            valid = band & (kl >= 0) & (kl < T_LOC) & (kg >= 0) & (kg < S)
            m = valid.astype(np.float32)
            for hs in range(2):
                maskM[:, t, 256 * hs:256 * hs + 128] = m[:128]
                maskM[0:64, t, 256 * hs + 128:256 * hs + 256] = m[128:]

        per_core.append({
            "srcT": srcT.astype(bf16),
            "pos_emb": pos_emb,
            "maskM": maskM.astype(bf16),
        })

    # constants
    shared["ident"] = np.eye(128, dtype=np.float32)

    flags = {}
    for l in range(L):
        for nm in ("bq", "bk", "bv", "bfc", "b1", "b2"):
            flags[f"{nm}{l}"] = not np.allclose(shared[f"{nm}{l}"], 0.0)
        for nm in ("ln1", "ln2"):
            flags[f"{nm}{l}"] = not (
                np.allclose(shared[f"{nm}g{l}"], 1.0)
                and np.allclose(shared[f"{nm}b{l}"], 0.0)
            )
    return shared, per_core, flags


def assemble(core_outs):
    out = np.zeros((B, S, H), np.float32)
    for c in range(NCORES):
        b, q = divmod(c, NCORES // B)
        out[b, q * CHUNK:(q + 1) * CHUNK] = core_outs[c]
    return out


# ---------------------------------------------------------------------------
# Bass program
# ---------------------------------------------------------------------------

def _legalize_waits(nc):
    """This container's walrus codegen accepts only ONE sync-wait per compute
    instruction ("Too many sync wait commands"). Tile's scheduler emits
    multi-wait instructions, so split: keep the last wait on the instruction
    and carry earlier ones on same-engine NoOps inserted right before it."""
    import concourse.mybir as mybir

    for fn in nc.m.functions:
        for blk in fn.blocks:
            out = []
            changed = False
            for inst in blk.instructions:
                si = getattr(inst, "sync_info", None)
                waits = list(si.on_wait) if si is not None and si.on_wait else []
                if len(waits) > 1 and not isinstance(
                        inst, mybir.InstEventSemaphore):
                    for j, w in enumerate(waits[:-1]):
                        # NoOp lowers through the v3 codegen only; Activation
                        # and Pool go through v2 (no InstISA nop) -> use a
                        # 1-wait Drain there instead.
                        if inst.engine in (mybir.EngineType.Activation,
                                           mybir.EngineType.Pool):
                            nop = mybir.InstDrain(
                                name=f"{inst.name}-w{j}", ins=[], outs=[])
                        else:
                            nop = mybir.InstNoOp(
                                name=f"{inst.name}-w{j}", ins=[], outs=[])
                        nop.engine = inst.engine
                        nop.sync_info = mybir.SyncInfo(on_wait=[w], on_update=[])
                        out.append(nop)
                    inst.sync_info = mybir.SyncInfo(
                        on_wait=[waits[-1]], on_update=list(si.on_update or []))
                    changed = True
                out.append(inst)
            if changed:
                blk.instructions = out


def build_program(flags):
    import concourse.bass as bass
    import concourse.mybir as mybir
    import concourse.tile as tile

    f32 = mybir.dt.float32
    bf = mybir.dt.bfloat16
    AF = mybir.ActivationFunctionType
    ALU = mybir.AluOpType

    nc = bass.Bass()
    FT = H // 128          # 6 feature tiles
    KTE = FK // 128        # 4 embedding contraction tiles

    # ---- DRAM tensors ----
    D = {}
    names = []

    def din(name, shape, dt):
        D[name] = nc.dram_tensor(name, shape, dt, kind="ExternalInput")
        names.append(name)

    din("srcT", [FK, T_LOC], bf)
    din("pos_emb", [T_LOC, H], f32)
    din("maskM", [128, NT, 512], bf)
    din("ident", [128, 128], f32)
    din("wembT", [FK, H], bf)
    for l in range(L):
        for nm in ("wqT", "wkT", "wvT", "wfcT", "w1T", "w2T"):
            din(f"{nm}{l}", [H, H], bf)
        for nm in ("bq", "bk", "bv", "bfc", "b1", "b2"):
            if flags[f"{nm}{l}"]:
                din(f"{nm}{l}", [H], f32)
        for nm in ("ln1", "ln2"):
            if flags[f"{nm}{l}"]:
                din(f"{nm}g{l}", [H], f32)
                din(f"{nm}b{l}", [H], f32)
    out_d = nc.dram_tensor("out", [CHUNK, H], f32, kind="ExternalOutput")

    def bcast_ap(dram, n):
        return bass.AP(tensor=dram.tensor, offset=dram.offset, ap=[[0, 128], [1, n]])

    with tile.TileContext(nc) as tc:
        import contextlib
        with contextlib.ExitStack() as ctx:
            consts = ctx.enter_context(tc.tile_pool(name="consts", bufs=1))
            acts = ctx.enter_context(tc.tile_pool(name="acts", bufs=1))
            work = ctx.enter_context(tc.tile_pool(name="work", bufs=2))
            ppool = ctx.enter_context(tc.tile_pool(name="pp", bufs=4))
            rpool = ctx.enter_context(tc.tile_pool(name="rp", bufs=2))
            spool = ctx.enter_context(tc.tile_pool(name="sp", bufs=4))
            # PSUM budget (8 banks): gemm [128,512]f32 1 bank x2, transpose
            # [128,512]f32 1 bank x2, attention scores 1 bank x2, ctx+den
            # 1 bank x2.
            psg = ctx.enter_context(tc.tile_pool(name="psg", bufs=2, space="PSUM"))
            pstr = ctx.enter_context(tc.tile_pool(name="pstr", bufs=1, space="PSUM"))
            pst = ctx.enter_context(tc.tile_pool(name="pst", bufs=3, space="PSUM"))
            psc = ctx.enter_context(tc.tile_pool(name="psc", bufs=2, space="PSUM"))

            # ---- constants / inputs to SBUF ----
            # srcT and the embedding weight are DMA'd first so the first
            # embedding matmuls are not queued behind the (larger) mask and
            # positional-table transfers.
            wpool = ctx.enter_context(tc.tile_pool(name="wpool", bufs=3))

            def load_w(name, kt=FT):
                wt = wpool.tile([128, kt, H], bf, name=f"{name}_sb", tag="wt")
                for k in range(kt):
                    nc.sync.dma_start(out=wt[:, k, :],
                                      in_=D[name][k * 128:(k + 1) * 128, :])
                return wt

            srcT_sb = consts.tile([128, KTE, T_LOC], bf)
            for kt in range(KTE):
                nc.sync.dma_start(out=srcT_sb[:, kt, :],
                                  in_=D["srcT"][kt * 128:(kt + 1) * 128, :])
            wembT_pre = load_w("wembT", kt=KTE)
            pos_sb = consts.tile([128, NT, H], f32)
            for t in range(NT):
                nc.sync.dma_start(out=pos_sb[:, t, :],
                                  in_=D["pos_emb"][t * 128:(t + 1) * 128, :])
            ident_sb = consts.tile([128, 128], f32)
            nc.sync.dma_start(out=ident_sb, in_=D["ident"][:, :])
            ones64 = consts.tile([128, 64], bf)
            nc.vector.memset(ones64, 1.0)
            eps_sb = consts.tile([128, 1], f32)
            nc.vector.memset(eps_sb, 1e-5)
            c768 = consts.tile([128, 1], f32)
            nc.vector.memset(c768, 1.0 / H)

            maskM_sb = consts.tile([128, NT, 512], bf)
            nc.sync.dma_start(out=maskM_sb, in_=D["maskM"][:, :, :])

            BIAS = {}
            for l in range(L):
                for nm in ("bq", "bk", "b1"):  # per-partition, feature-major
                    if flags[f"{nm}{l}"]:
                        BIAS[f"{nm}{l}"] = consts.tile([128, FT], f32, name=f"{nm}{l}_sb")
                        nc.sync.dma_start(
                            out=BIAS[f"{nm}{l}"],
                            in_=D[f"{nm}{l}"].rearrange("(kt p) -> p kt", p=128))
                for nm in ("bv", "bfc", "b2"):  # broadcast, token-major
                    if flags[f"{nm}{l}"]:
                        BIAS[f"{nm}{l}"] = consts.tile([128, H], f32, name=f"{nm}{l}_sb")
                        nc.sync.dma_start(
                            out=BIAS[f"{nm}{l}"], in_=bcast_ap(D[f"{nm}{l}"], H))
                for nm in ("ln1", "ln2"):
                    if flags[f"{nm}{l}"]:
                        for gb in ("g", "b"):
                            BIAS[f"{nm}{gb}{l}"] = consts.tile([128, H], f32, name=f"{nm}{gb}{l}_sb")
                            nc.sync.dma_start(
                                out=BIAS[f"{nm}{gb}{l}"],
                                in_=bcast_ap(D[f"{nm}{gb}{l}"], H))

            # ---- persistent activations ----
            x_tok = acts.tile([128, NT, H], f32)          # token-major f32
            qT = acts.tile([128, FT, T_LOC], bf)
            kTp = acts.tile([128, FT, KPAD], bf)
            V_sh = acts.tile([128, FT, H], bf)            # 6 shifted token tiles
            ctxT = acts.tile([128, FT, T_LOC], bf)
            H1T = acts.tile([128, FT, T_LOC], bf)

            # xT: feature-major bf16 with 32-col zero pad on each side (cols
            # [32, 672) hold tokens [0, 640)); a fresh generation per
            # transpose-set so the pool tracks lifetimes.
            xtp = ctx.enter_context(tc.tile_pool(name="xtp", bufs=2))

            def new_xT(name):
                t_ = xtp.tile([128, FT, H], bf, name=name, tag="xTslot")
                nc.vector.memset(t_[:, :, 0:32], 0.0)
                nc.vector.memset(t_[:, :, 32 + T_LOC:H], 0.0)
                return t_

            # ---- embedding ----
            # [128,640] f32 psum tiles span 2 banks; each matmul output
            # (N=512 then N=128) stays inside one bank. The two chunks share
            # one LDWEIGHTS per contraction tile.
            wembT_sb = wembT_pre
            for t in range(NT):
                for c0, cn in ((0, 384), (384, 384)):
                    ps = psg.tile([128, 512], f32, tag="gemm")
                    for kt in range(KTE):
                        nc.tensor.matmul(ps[:, 0:cn],
                                         srcT_sb[:, kt, t * 128:(t + 1) * 128],
                                         wembT_sb[:, kt, c0:c0 + cn],
                                         start=(kt == 0), stop=(kt == KTE - 1))
                    nc.vector.tensor_add(x_tok[:, t, c0:c0 + cn], ps[:, 0:cn],
                                         pos_sb[:, t, c0:c0 + cn])

            def transpose_set(dst, t):
                """PE-transpose x_tok tile t into dst[:, :, 128t:+128] (bf16)."""
                for g in range(2):
                    n_g = 4 if g == 0 else 2
                    trp = pstr.tile([128, 512], f32, tag="tr")
                    for j in range(n_g):
                        ft = 4 * g + j
                        nc.tensor.transpose(
                            trp[:, j * 128:(j + 1) * 128],
                            x_tok[:, t, ft * 128:(ft + 1) * 128], ident_sb)
                    src = trp[:, 0:n_g * 128].rearrange("p (a b) -> p a b", b=128)
                    nc.vector.tensor_copy(
                        dst[:, 4 * g:4 * g + n_g, 32 + t * 128:32 + (t + 1) * 128],
                        src)

            xT = new_xT("x0T")
            for t in range(NT):
                transpose_set(xT, t)

            # kTp pads are written once; the per-layer k GEMM only fills
            # cols [32, 672) so the pads stay zero across layers.
            nc.vector.memset(kTp[:, :, 0:32], 0.0)
            nc.vector.memset(kTp[:, :, 32 + T_LOC:KPAD], 0.0)

            # ---- layers ----
            for l in range(L):

                # q/k GEMMs (feature-major outputs); the {512,128} token
                # chunks accumulate in separate banks of one [128,640] psum
                # tile and share one LDWEIGHTS per contraction tile.
                wq = load_w(f"wqT{l}")
                wk = load_w(f"wkT{l}")
                for ft in range(FT):
                    for wm, dst, bflag in (
                            (wq, qT[:, ft, 0:T_LOC], f"bq{l}"),
                            (wk, kTp[:, ft, 32:32 + T_LOC], f"bk{l}")):
                        for c0, cn in ((0, 320), (320, 320)):
                            ps = psg.tile([128, 512], f32, tag="gemm")
                            for kt in range(FT):
                                nc.tensor.matmul(
                                    ps[:, 0:cn],
                                    wm[:, kt, ft * 128:(ft + 1) * 128],
                                    xT[:, kt, 32 + c0:32 + c0 + cn],
                                    start=(kt == 0), stop=(kt == FT - 1))
                            if flags[bflag]:
                                nc.vector.tensor_scalar_add(
                                    dst[:, c0:c0 + cn], ps[:, 0:cn],
                                    BIAS[bflag][:, ft:ft + 1])
                            else:
                                nc.vector.tensor_copy(dst[:, c0:c0 + cn],
                                                      ps[:, 0:cn])

                # V GEMM for shifted window i, emitted just-in-time from
                # inside the attention loop so the PE keeps dense matmul
                # work during the exp/mask waits (HAM stays un-throttled).
                wv = load_w(f"wvT{l}")

                def v_window(i, wv=wv, l=l, xT=xT):
                    for c0, cn in ((0, 384), (384, 384)):
                        ps = psg.tile([128, 512], f32, tag="gemm")
                        for kt in range(FT):
                            nc.tensor.matmul(
                                ps[:, 0:cn], xT[:, kt, 128 * i:128 * i + 128],
                                wv[:, kt, c0:c0 + cn],
                                start=(kt == 0), stop=(kt == FT - 1))
                        if flags[f"bv{l}"]:
                            nc.vector.tensor_add(
                                V_sh[:, i, c0:c0 + cn], ps[:, 0:cn],
                                BIAS[f"bv{l}"][:, c0:c0 + cn])
                        else:
                            nc.vector.tensor_copy(V_sh[:, i, c0:c0 + cn],
                                                  ps[:, 0:cn])

                v_window(0)
                v_window(1)

                # attention: unnormalized ctx + denominators; the softmax
                # divide is deferred to one batched ln/exp pass per layer
                # (keeps the ACT engine on a single table set all kernel).
                den_t = acts.tile([128, NT, FT * 128], bf,
                                  name=f"den{l}", tag="den")
                wfc = load_w(f"wfcT{l}")
                w1 = load_w(f"w1T{l}")
                xTn = new_xT(f"x1T{l}")

                def w1_chunk(c0, cn, w1=w1, xT=xTn, l=l):
                    for ft in range(FT):
                        bias = (BIAS[f"b1{l}"][:, ft:ft + 1]
                                if flags[f"b1{l}"] else 0.0)
                        ps = psg.tile([128, 512], f32, tag="gemm")
                        for kt in range(FT):
                            nc.tensor.matmul(
                                ps[:, 0:cn], w1[:, kt, ft * 128:(ft + 1) * 128],
                                xT[:, kt, 32 + c0:32 + c0 + cn],
                                start=(kt == 0), stop=(kt == FT - 1))
                        nc.scalar.activation(
                            H1T[:, ft, c0:c0 + cn], ps[:, 0:cn], AF.Relu,
                            bias=bias)

                def emit_tail(t, l=l, wfc=wfc, xTn=xTn, den_t=den_t):
                    # softmax divide rb = exp(-ln(den)), normalize ctx, then
                    # fc + residual + LN1 + transpose for tile t.
                    nc.vector.tensor_scalar_max(
                        den_t[:, t, :], den_t[:, t, :], 1e-6)
                    lnv = rpool.tile([128, T_LOC + 128], f32, tag="lnv")
                    nc.scalar.activation(lnv[:, 0:768], den_t[:, t, :], AF.Ln)
                    rb = rpool.tile([128, T_LOC + 128], bf, tag="rb")
                    nc.scalar.activation(rb[:, 0:768], lnv[:, 0:768],
                                         AF.Exp, scale=-1.0)
                    rb_r = rb[:, 0:768].rearrange("p (a b) -> p a b", b=128)
                    nc.vector.tensor_tensor(
                        out=ctxT[:, 0:FT, 128 * t:128 * t + 128],
                        in0=ctxT[:, 0:FT, 128 * t:128 * t + 128],
                        in1=rb_r, op=ALU.mult)
                    F = work.tile([128, H], f32, tag="F")
                    s1 = spool.tile([128, 2], f32, tag="s1")
                    for j, (c0, cn) in enumerate(((0, 384), (384, 384))):
                        ps = psg.tile([128, 512], f32, tag="gemm")
                        for kt in range(FT):
                            nc.tensor.matmul(
                                ps[:, 0:cn], ctxT[:, kt, 128 * t:128 * t + 128],
                                wfc[:, kt, c0:c0 + cn],
                                start=(kt == 0), stop=(kt == FT - 1))
                        nc.vector.scalar_tensor_tensor(
                            out=F[:, c0:c0 + cn], in0=ps[:, 0:cn], scalar=1.0,
                            in1=x_tok[:, t, c0:c0 + cn], op0=ALU.mult,
                            op1=ALU.add, accum_out=s1[:, j:j + 1])
                    if flags[f"bfc{l}"]:
                        nc.vector.tensor_add(F, F, BIAS[f"bfc{l}"])
                    _layernorm(nc, work, spool, F, s1, x_tok, t, eps_sb, c768,
                               BIAS.get(f"ln1g{l}"), BIAS.get(f"ln1b{l}"),
                               f32, AF, ALU)
                    transpose_set(xTn, t)

                for t in range(NT):
                    if t + 2 < FT:
                        v_window(t + 2)
                    for hp in range(FT):
                        cps = psc.tile([128, 256], f32, tag="ctx")
                        for hs in range(2):
                            h = 2 * hp + hs
                            po = 64 * hs
                            stp = pst.tile([128, 256], f32, tag="st")
                            nc.tensor.matmul(
                                stp[:, 0:128],
                                kTp[po:po + 64, hp, 128 * t:128 * t + 128],
                                qT[po:po + 64, hp, 128 * t:128 * t + 128],
                                start=True, stop=True)
                            nc.tensor.matmul(
                                stp[:, 128:256],
                                kTp[po:po + 64, hp, 128 * t + 128:128 * t + 256],
                                qT[po:po + 64, hp, 128 * t:128 * t + 128],
                                start=True, stop=True)
                            p_full = ppool.tile([128, 256], bf, tag="p_full")
                            nc.scalar.activation(p_full, stp[:, 0:256], AF.Exp)
                            nc.vector.tensor_tensor(
                                out=p_full, in0=p_full,
                                in1=maskM_sb[:, t, 0:256], op=ALU.mult)
                            nc.tensor.matmul(
                                cps[po:po + 64, 128:256], ones64[0:128, :],
                                p_full[:, 0:128], start=True, stop=False)
                            nc.tensor.matmul(
                                cps[po:po + 64, 128:256], ones64[0:64, :],
                                p_full[0:64, 128:256], start=False, stop=True)
                            nc.tensor.matmul(
                                cps[po:po + 64, 0:128],
                                V_sh[0:128, t, 64 * h:64 * h + 64],
                                p_full[:, 0:128], start=True, stop=False)
                            nc.tensor.matmul(
                                cps[po:po + 64, 0:128],
                                V_sh[0:64, t + 1, 64 * h:64 * h + 64],
                                p_full[0:64, 128:256], start=False, stop=True)
                        nc.vector.tensor_copy(
                            ctxT[:, hp, 128 * t:128 * t + 128], cps[:, 0:128])
                        nc.vector.tensor_copy(
                            den_t[:, t, 128 * hp:128 * hp + 128],
                            cps[:, 128:256])

                    # tail of the PREVIOUS tile: emitted after this
                    # tile's scores/ctx so its recip/LN chain never
                    # head-of-line blocks the PE FIFO.
                    if t == NT - 1:
                        w1_chunk(0, 384)
                    if t > 0:
                        emit_tail(t - 1)
                emit_tail(NT - 1)

                xT = xTn

                # FFN: the W1 token chunks for tokens 384+ are emitted
                # from inside the W2/LN2 tile loop (the 0:384 chunk already
                # ran inside the attention loop).
                w2 = load_w(f"w2T{l}")
                if l < L - 1:
                    xT = new_xT(f"x2T{l}")
                for t in range(NT):
                    if t < 2:
                        w1_chunk(384 + 128 * t, 128)
                    F2 = work.tile([128, H], f32, tag="F")
                    s1 = spool.tile([128, 2], f32, tag="s1")
                    for j, (c0, cn) in enumerate(((0, 384), (384, 384))):
                        ps = psg.tile([128, 512], f32, tag="gemm")
                        for kt in range(FT):
                            nc.tensor.matmul(
                                ps[:, 0:cn], H1T[:, kt, 128 * t:128 * t + 128],
                                w2[:, kt, c0:c0 + cn],
                                start=(kt == 0), stop=(kt == FT - 1))
                        nc.vector.scalar_tensor_tensor(
                            out=F2[:, c0:c0 + cn], in0=ps[:, 0:cn], scalar=1.0,
                            in1=x_tok[:, t, c0:c0 + cn], op0=ALU.mult,
                            op1=ALU.add, accum_out=s1[:, j:j + 1])
                    if flags[f"b2{l}"]:
                        nc.vector.tensor_add(F2, F2, BIAS[f"b2{l}"])
                    _layernorm(nc, work, spool, F2, s1, x_tok, t, eps_sb, c768,
                               BIAS.get(f"ln2g{l}"), BIAS.get(f"ln2b{l}"),
                               f32, AF, ALU)
                    if l < L - 1:
                        if t > 0:
                            transpose_set(xT, t - 1)
                        if t == NT - 1:
                            transpose_set(xT, t)
                    else:
                        lo = max(128 * t, HALO) - 128 * t
                        hi = min(128 * t + 128, HALO + CHUNK) - 128 * t
                        nc.sync.dma_start(
                            out=out_d[128 * t + lo - HALO:128 * t + hi - HALO, :],
                            in_=x_tok[lo:hi, t, :])

    _legalize_waits(nc)
    return nc, names


def _layernorm(nc, work, spool, F, s1, x_tok, t, eps_sb, c768,
               g_bc, b_bc, f32, AF, ALU):
    import ml_dtypes as _md  # noqa: F401
    # mean from the residual-add's accumulated row-sums; E[x^2] from an ACT
    # Square pass with accum_out (moves all stats work off the vector
    # engine). rstd = exp(-0.5*ln(var+eps)) stays on the one ACT table set.
    fsq = work.tile([128, 768], F.dtype if hasattr(F, 'dtype') else f32,
                    name="fsq", tag="fsq")
    s2 = spool.tile([128, 1], f32, tag="s2")
    nc.scalar.activation(fsq, F, AF.Square, accum_out=s2)
    mean = spool.tile([128, 1], f32, tag="mean")
    nc.vector.scalar_tensor_tensor(
        out=mean, in0=s1[:, 0:1], scalar=s1[:, 1:2], in1=c768[:, 0:1],
        op0=ALU.add, op1=ALU.mult)
    m2 = spool.tile([128, 1], f32, tag="m2")
    nc.vector.tensor_scalar_mul(m2, mean, mean[:, 0:1])
    var = spool.tile([128, 1], f32, tag="var")
    nc.vector.scalar_tensor_tensor(
        out=var, in0=s2, scalar=1.0 / 768.0, in1=m2,
        op0=ALU.mult, op1=ALU.subtract)
    lv = spool.tile([128, 1], f32, tag="lv")
    nc.scalar.activation(lv, var, AF.Ln, bias=eps_sb[:, 0:1])
    rstd = spool.tile([128, 1], f32, tag="rstd")
    nc.scalar.activation(rstd, lv, AF.Exp, scale=-0.5)
    nc.vector.tensor_scalar(
        out=x_tok[:, t, :], in0=F, scalar1=mean, scalar2=rstd,
        op0=ALU.subtract, op1=ALU.mult)
    if g_bc is not None:
        nc.vector.tensor_tensor(
            out=x_tok[:, t, :], in0=x_tok[:, t, :], in1=g_bc, op=ALU.mult)
        nc.vector.tensor_tensor(
            out=x_tok[:, t, :], in0=x_tok[:, t, :], in1=b_bc, op=ALU.add)


def run_on_device(shared, per_core, flags, trace=False):
    from concourse.bass_utils import run_bass_kernel_spmd

    nc, names = build_program(flags)
    in_maps = []
    for c in range(NCORES):
        m = {}
        for n in names:
            src = per_core[c] if n in per_core[c] else shared
            m[n] = np.ascontiguousarray(src[n])
        in_maps.append(m)
    res = run_bass_kernel_spmd(nc, in_maps, core_ids=list(range(NCORES)),
                               trace=trace)
    return [r["out"] for r in res.results], res


def kernel(**inputs):
    shared, per_core, flags = host_prep(inputs)
    core_outs, _ = run_on_device(shared, per_core, flags)
    return assemble(core_outs)




# revision 40
# speedup vs baseline: 1.0289x; 1.0289x over previous
"""Trainium2 Bass kernel for a 2-layer Longformer-style encoder.

Model: B=2, S=2048, F=438, H=768, NH=12, HD=64, one-sided window w=32, L=2.

Sharding: 8 cores, data-parallel over (batch, sequence-quarter). Each core
computes 512 output tokens from a 640-token local window (64-token halo on
each side covers the 2-layer receptive field), so no collectives are needed.

Device algorithm per core (uniform SPMD, 640 local tokens):
  - x0 = srcT_pad.T @ W_embT + (pos_emb + b_emb)           [token-major f32]
  - per layer:
      xT   = transpose(x)  bf16                             [feature-major]
      qT   = W_qT'.T @ xT, scaled by HD^-0.5 on host        [feature-major]
      kTp  = W_kT.T @ xT at free-offset 32 in a 768-wide
             zero-padded buffer                             [feature-major]
      per query tile t (software-pipelined; V window t+2 is
      GEMMed just-in-time to keep the PE dense/warm):
        per head h: ST = banded score matmuls               [n-major scores]
        P = exp(ST) * mask01  (post-exp multiplicative mask; zero-padded
            keys keep the un-masked scores finite)
        den[h,q] += ones-matmuls over P (replicated per head row-block)
        ctxT[64h:+64, t] += V_sh.T @ P                      [feature-major]
        rb = exp(-ln(max(den,1e-6)))  (reciprocal via Ln/Exp so the ACT
            engine never leaves the natural_log_exp table set)
        ctxT *= rb; fc: F = ctxT.T @ W_fcT + residual
        LN1 (rstd = exp(-0.5 ln(var+eps))) -> x1, transpose -> x1T
      H1T = relu(W_1T.T @ x1T)                              [feature-major]
      F2 = H1T.T @ W_2T + x1; LN2 -> x2
  - out = x2[64:576]
"""

import numpy as np
import ml_dtypes

B, S, F_DIM, H, NH, HD, W_ONE, L = 2, 2048, 438, 768, 12, 64, 32, 2
NCORES = 8
CHUNK = 512          # output tokens per core
HALO = 64            # per side
T_LOC = CHUNK + 2 * HALO   # 640 local tokens
NT = T_LOC // 128          # 5 query tiles
KPAD = T_LOC + 128         # 768 padded key width (full 128-wide second score block)
SPAN = 192                 # keys per query tile (128 + 2*32)
FK = 512                   # padded embedding contraction (438 -> 512)
# Large enough that masked positions contribute ~e^-50 ~ 2e-22 (negligible),
# small enough that a fully-masked (pad) query row keeps a nonzero softmax
# denominator -> no inf/NaN anywhere (pad rows are discarded on output).
MASK_NEG = -50.0

bf16 = ml_dtypes.bfloat16


def _np(x):
    return np.asarray(x)


def host_prep(inputs):
    """Split full inputs into shared weight arrays + per-core arrays."""
    src_seq = _np(inputs["src_seq"]).astype(np.float32)
    src_pos = _np(inputs["src_pos"]).astype(np.int32)
    pos_table = _np(inputs["pos_table"]).astype(np.float32)

    shared = {}
    qscale = float(HD) ** -0.5

    W_emb = _np(inputs["W_emb"]).astype(np.float32)        # [H, F]
    WembT = np.zeros((FK, H), np.float32)
    WembT[:F_DIM] = W_emb.T
    shared["wembT"] = WembT.astype(bf16)

    for l in range(L):
        Wq = _np(inputs["Wq"])[l].astype(np.float32)
        Wk = _np(inputs["Wk"])[l].astype(np.float32)
        Wv = _np(inputs["Wv"])[l].astype(np.float32)
        Wfc = _np(inputs["Wfc"])[l].astype(np.float32)
        W1 = _np(inputs["W1"])[l].astype(np.float32)
        W2 = _np(inputs["W2"])[l].astype(np.float32)
        shared[f"wqT{l}"] = (Wq.T * qscale).astype(bf16)   # [H_in, H_out]
        shared[f"wkT{l}"] = Wk.T.astype(bf16)
        shared[f"wvT{l}"] = Wv.T.astype(bf16)
        shared[f"wfcT{l}"] = Wfc.T.astype(bf16)
        shared[f"w1T{l}"] = W1.T.astype(bf16)
        shared[f"w2T{l}"] = W2.T.astype(bf16)
        shared[f"bq{l}"] = (_np(inputs["bq"])[l].astype(np.float32) * qscale)
        shared[f"bk{l}"] = _np(inputs["bk"])[l].astype(np.float32)
        shared[f"bv{l}"] = _np(inputs["bv"])[l].astype(np.float32)
        shared[f"bfc{l}"] = _np(inputs["bfc"])[l].astype(np.float32)
        shared[f"b1{l}"] = _np(inputs["b1"])[l].astype(np.float32)
        shared[f"b2{l}"] = _np(inputs["b2"])[l].astype(np.float32)
        shared[f"ln1g{l}"] = _np(inputs["ln1_g"])[l].astype(np.float32)
        shared[f"ln1b{l}"] = _np(inputs["ln1_b"])[l].astype(np.float32)
        shared[f"ln2g{l}"] = _np(inputs["ln2_g"])[l].astype(np.float32)
        shared[f"ln2b{l}"] = _np(inputs["ln2_b"])[l].astype(np.float32)

    b_emb = _np(inputs["b_emb"]).astype(np.float32)

    per_core = []
    for c in range(NCORES):
        b, q = divmod(c, NCORES // B)
        gstart = q * CHUNK - HALO
        lo, hi = max(gstart, 0), min(gstart + T_LOC, S)

        src_halo = np.zeros((T_LOC, F_DIM), np.float32)
        src_halo[lo - gstart: hi - gstart] = src_seq[b, lo:hi]
        srcT = np.zeros((FK, T_LOC), np.float32)
        srcT[:F_DIM] = src_halo.T

        pos_emb = np.zeros((T_LOC, H), np.float32)
        pos_emb[lo - gstart: hi - gstart] = pos_table[src_pos[b, lo:hi]]
        pos_emb += b_emb[None, :]

        # n-major multiplicative 0/1 mask per query tile, applied to
        # P = exp(scores) AFTER the exp: maskM[n, t, 0:128] covers span keys
        # 0..127, maskM[0:64, t, 128:256] covers span keys 128..191 (rows
        # 64:128 of the second block are always 0 - those score rows are
        # computed but unused). local key = 128t - 32 + n, query = 128t + q.
        maskM = np.zeros((128, NT, 512), np.float32)
        for t in range(NT):
            n = np.arange(SPAN)[:, None]
            qq = np.arange(128)[None, :]
            kl = 128 * t - 32 + n
            kg = gstart + kl
            band = np.abs(kl - (128 * t + qq)) <= W_ONE
            valid = band & (kl >= 0) & (kl < T_LOC) & (kg >= 0) & (kg < S)
            m = valid.astype(np.float32)
            for hs in range(2):
                maskM[:, t, 256 * hs:256 * hs + 128] = m[:128]
                maskM[0:64, t, 256 * hs + 128:256 * hs + 256] = m[128:]

        per_core.append({
            "srcT": srcT.astype(bf16),
            "pos_emb": pos_emb,
            "maskM": maskM.astype(bf16),
        })

    # constants
    shared["ident"] = np.eye(128, dtype=np.float32)

    flags = {}
    for l in range(L):
        for nm in ("bq", "bk", "bv", "bfc", "b1", "b2"):
            flags[f"{nm}{l}"] = not np.allclose(shared[f"{nm}{l}"], 0.0)
        for nm in ("ln1", "ln2"):
            flags[f"{nm}{l}"] = not (
                np.allclose(shared[f"{nm}g{l}"], 1.0)
                and np.allclose(shared[f"{nm}b{l}"], 0.0)
            )
    return shared, per_core, flags


def assemble(core_outs):
    out = np.zeros((B, S, H), np.float32)
    for c in range(NCORES):
        b, q = divmod(c, NCORES // B)
        out[b, q * CHUNK:(q + 1) * CHUNK] = core_outs[c]
    return out


# ---------------------------------------------------------------------------
# Bass program
# ---------------------------------------------------------------------------

def _legalize_waits(nc):
    """This container's walrus codegen accepts only ONE sync-wait per compute
    instruction ("Too many sync wait commands"). Tile's scheduler emits
    multi-wait instructions, so split: keep the last wait on the instruction
    and carry earlier ones on same-engine NoOps inserted right before it."""
    import concourse.mybir as mybir

    for fn in nc.m.functions:
        for blk in fn.blocks:
            out = []
            changed = False
            for inst in blk.instructions:
                si = getattr(inst, "sync_info", None)
                waits = list(si.on_wait) if si is not None and si.on_wait else []
                if len(waits) > 1 and not isinstance(
                        inst, mybir.InstEventSemaphore):
                    for j, w in enumerate(waits[:-1]):
                        # NoOp lowers through the v3 codegen only; Activation
                        # and Pool go through v2 (no InstISA nop) -> use a
                        # 1-wait Drain there instead.
                        if inst.engine in (mybir.EngineType.Activation,
                                           mybir.EngineType.Pool):
                            nop = mybir.InstDrain(
                                name=f"{inst.name}-w{j}", ins=[], outs=[])
                        else:
                            nop = mybir.InstNoOp(
                                name=f"{inst.name}-w{j}", ins=[], outs=[])
                        nop.engine = inst.engine
                        nop.sync_info = mybir.SyncInfo(on_wait=[w], on_update=[])
                        out.append(nop)
                    inst.sync_info = mybir.SyncInfo(
                        on_wait=[waits[-1]], on_update=list(si.on_update or []))
                    changed = True
                out.append(inst)
            if changed:
                blk.instructions = out


def build_program(flags):
    import concourse.bass as bass
    import concourse.mybir as mybir
    import concourse.tile as tile

    f32 = mybir.dt.float32
    bf = mybir.dt.bfloat16
    AF = mybir.ActivationFunctionType
    ALU = mybir.AluOpType

    nc = bass.Bass()
    FT = H // 128          # 6 feature tiles
    KTE = FK // 128        # 4 embedding contraction tiles

    # ---- DRAM tensors ----
    D = {}
    names = []

    def din(name, shape, dt):
        D[name] = nc.dram_tensor(name, shape, dt, kind="ExternalInput")
        names.append(name)

    din("srcT", [FK, T_LOC], bf)
    din("pos_emb", [T_LOC, H], f32)
    din("maskM", [128, NT, 512], bf)
    din("ident", [128, 128], f32)
    din("wembT", [FK, H], bf)
    for l in range(L):
        for nm in ("wqT", "wkT", "wvT", "wfcT", "w1T", "w2T"):
            din(f"{nm}{l}", [H, H], bf)
        for nm in ("bq", "bk", "bv", "bfc", "b1", "b2"):
            if flags[f"{nm}{l}"]:
                din(f"{nm}{l}", [H], f32)
        for nm in ("ln1", "ln2"):
            if flags[f"{nm}{l}"]:
                din(f"{nm}g{l}", [H], f32)
                din(f"{nm}b{l}", [H], f32)
    out_d = nc.dram_tensor("out", [CHUNK, H], f32, kind="ExternalOutput")

    def bcast_ap(dram, n):
        return bass.AP(tensor=dram.tensor, offset=dram.offset, ap=[[0, 128], [1, n]])

    with tile.TileContext(nc) as tc:
        import contextlib
        with contextlib.ExitStack() as ctx:
            consts = ctx.enter_context(tc.tile_pool(name="consts", bufs=1))
            acts = ctx.enter_context(tc.tile_pool(name="acts", bufs=1))
            work = ctx.enter_context(tc.tile_pool(name="work", bufs=2))
            ppool = ctx.enter_context(tc.tile_pool(name="pp", bufs=4))
            rpool = ctx.enter_context(tc.tile_pool(name="rp", bufs=2))
            spool = ctx.enter_context(tc.tile_pool(name="sp", bufs=4))
            # PSUM budget (8 banks): gemm [128,512]f32 1 bank x2, transpose
            # [128,512]f32 1 bank x2, attention scores 1 bank x2, ctx+den
            # 1 bank x2.
            psg = ctx.enter_context(tc.tile_pool(name="psg", bufs=2, space="PSUM"))
            pstr = ctx.enter_context(tc.tile_pool(name="pstr", bufs=1, space="PSUM"))
            pst = ctx.enter_context(tc.tile_pool(name="pst", bufs=3, space="PSUM"))
            psc = ctx.enter_context(tc.tile_pool(name="psc", bufs=2, space="PSUM"))

            # ---- constants / inputs to SBUF ----
            # srcT and the embedding weight are DMA'd first so the first
            # embedding matmuls are not queued behind the (larger) mask and
            # positional-table transfers.
            wpool = ctx.enter_context(tc.tile_pool(name="wpool", bufs=4))

            def load_w(name, kt=FT):
                wt = wpool.tile([128, kt, H], bf, name=f"{name}_sb", tag="wt")
                for k in range(kt):
                    nc.sync.dma_start(out=wt[:, k, :],
                                      in_=D[name][k * 128:(k + 1) * 128, :])
                return wt

            srcT_sb = consts.tile([128, KTE, T_LOC], bf)
            for kt in range(KTE):
                nc.sync.dma_start(out=srcT_sb[:, kt, :],
                                  in_=D["srcT"][kt * 128:(kt + 1) * 128, :])
            wembT_pre = load_w("wembT", kt=KTE)
            pos_sb = consts.tile([128, NT, H], f32)
            for t in range(NT):
                nc.sync.dma_start(out=pos_sb[:, t, :],
                                  in_=D["pos_emb"][t * 128:(t + 1) * 128, :])
            ident_sb = consts.tile([128, 128], f32)
            nc.sync.dma_start(out=ident_sb, in_=D["ident"][:, :])
            ones64 = consts.tile([128, 64], bf)
            nc.vector.memset(ones64, 1.0)
            eps_sb = consts.tile([128, 1], f32)
            nc.vector.memset(eps_sb, 1e-5)
            c768 = consts.tile([128, 1], f32)
            nc.vector.memset(c768, 1.0 / H)

            maskM_sb = consts.tile([128, NT, 512], bf)
            nc.sync.dma_start(out=maskM_sb, in_=D["maskM"][:, :, :])

            BIAS = {}
            for l in range(L):
                for nm in ("bq", "bk", "b1"):  # per-partition, feature-major
                    if flags[f"{nm}{l}"]:
                        BIAS[f"{nm}{l}"] = consts.tile([128, FT], f32, name=f"{nm}{l}_sb")
                        nc.sync.dma_start(
                            out=BIAS[f"{nm}{l}"],
                            in_=D[f"{nm}{l}"].rearrange("(kt p) -> p kt", p=128))
                for nm in ("bv", "bfc", "b2"):  # broadcast, token-major
                    if flags[f"{nm}{l}"]:
                        BIAS[f"{nm}{l}"] = consts.tile([128, H], f32, name=f"{nm}{l}_sb")
                        nc.sync.dma_start(
                            out=BIAS[f"{nm}{l}"], in_=bcast_ap(D[f"{nm}{l}"], H))
                for nm in ("ln1", "ln2"):
                    if flags[f"{nm}{l}"]:
                        for gb in ("g", "b"):
                            BIAS[f"{nm}{gb}{l}"] = consts.tile([128, H], f32, name=f"{nm}{gb}{l}_sb")
                            nc.sync.dma_start(
                                out=BIAS[f"{nm}{gb}{l}"],
                                in_=bcast_ap(D[f"{nm}{gb}{l}"], H))

            # ---- persistent activations ----
            x_tok = acts.tile([128, NT, H], f32)          # token-major f32
            qT = acts.tile([128, FT, T_LOC], bf)
            kTp = acts.tile([128, FT, KPAD], bf)
            V_sh = acts.tile([128, FT, H], bf)            # 6 shifted token tiles
            ctxT = acts.tile([128, FT, T_LOC], bf)
            H1T = acts.tile([128, FT, T_LOC], bf)

            # xT: feature-major bf16 with 32-col zero pad on each side (cols
            # [32, 672) hold tokens [0, 640)); a fresh generation per
            # transpose-set so the pool tracks lifetimes.
            xtp = ctx.enter_context(tc.tile_pool(name="xtp", bufs=2))

            def new_xT(name):
                t_ = xtp.tile([128, FT, H], bf, name=name, tag="xTslot")
                nc.vector.memset(t_[:, :, 0:32], 0.0)
                nc.vector.memset(t_[:, :, 32 + T_LOC:H], 0.0)
                return t_

            # ---- embedding ----
            # [128,640] f32 psum tiles span 2 banks; each matmul output
            # (N=512 then N=128) stays inside one bank. The two chunks share
            # one LDWEIGHTS per contraction tile.
            wembT_sb = wembT_pre
            for t in range(NT):
                for c0, cn in ((0, 384), (384, 384)):
                    ps = psg.tile([128, 512], f32, tag="gemm")
                    for kt in range(KTE):
                        nc.tensor.matmul(ps[:, 0:cn],
                                         srcT_sb[:, kt, t * 128:(t + 1) * 128],
                                         wembT_sb[:, kt, c0:c0 + cn],
                                         start=(kt == 0), stop=(kt == KTE - 1))
                    nc.vector.tensor_add(x_tok[:, t, c0:c0 + cn], ps[:, 0:cn],
                                         pos_sb[:, t, c0:c0 + cn])

            def transpose_set(dst, t):
                """PE-transpose x_tok tile t into dst[:, :, 128t:+128] (bf16)."""
                for g in range(2):
                    n_g = 4 if g == 0 else 2
                    trp = pstr.tile([128, 512], f32, tag="tr")
                    for j in range(n_g):
                        ft = 4 * g + j
                        nc.tensor.transpose(
                            trp[:, j * 128:(j + 1) * 128],
                            x_tok[:, t, ft * 128:(ft + 1) * 128], ident_sb)
                    src = trp[:, 0:n_g * 128].rearrange("p (a b) -> p a b", b=128)
                    nc.vector.tensor_copy(
                        dst[:, 4 * g:4 * g + n_g, 32 + t * 128:32 + (t + 1) * 128],
                        src)

            xT = new_xT("x0T")
            for t in range(NT):
                transpose_set(xT, t)

            # kTp pads are written once; the per-layer k GEMM only fills
            # cols [32, 672) so the pads stay zero across layers.
            nc.vector.memset(kTp[:, :, 0:32], 0.0)
            nc.vector.memset(kTp[:, :, 32 + T_LOC:KPAD], 0.0)

            # ---- layers ----
            for l in range(L):

                # q/k GEMMs (feature-major outputs); the {512,128} token
                # chunks accumulate in separate banks of one [128,640] psum
                # tile and share one LDWEIGHTS per contraction tile.
                wq = load_w(f"wqT{l}")
                wk = load_w(f"wkT{l}")
                for ft in range(FT):
                    for wm, dst, bflag in (
                            (wq, qT[:, ft, 0:T_LOC], f"bq{l}"),
                            (wk, kTp[:, ft, 32:32 + T_LOC], f"bk{l}")):
                        for c0, cn in ((0, 320), (320, 320)):
                            ps = psg.tile([128, 512], f32, tag="gemm")
                            for kt in range(FT):
                                nc.tensor.matmul(
                                    ps[:, 0:cn],
                                    wm[:, kt, ft * 128:(ft + 1) * 128],
                                    xT[:, kt, 32 + c0:32 + c0 + cn],
                                    start=(kt == 0), stop=(kt == FT - 1))
                            if flags[bflag]:
                                nc.vector.tensor_scalar_add(
                                    dst[:, c0:c0 + cn], ps[:, 0:cn],
                                    BIAS[bflag][:, ft:ft + 1])
                            else:
                                nc.vector.tensor_copy(dst[:, c0:c0 + cn],
                                                      ps[:, 0:cn])

                # V GEMM for shifted window i, emitted just-in-time from
                # inside the attention loop so the PE keeps dense matmul
                # work during the exp/mask waits (HAM stays un-throttled).
                wv = load_w(f"wvT{l}")

                def v_window(i, wv=wv, l=l, xT=xT):
                    for c0, cn in ((0, 384), (384, 384)):
                        ps = psg.tile([128, 512], f32, tag="gemm")
                        for kt in range(FT):
                            nc.tensor.matmul(
                                ps[:, 0:cn], xT[:, kt, 128 * i:128 * i + 128],
                                wv[:, kt, c0:c0 + cn],
                                start=(kt == 0), stop=(kt == FT - 1))
                        if flags[f"bv{l}"]:
                            nc.vector.tensor_add(
                                V_sh[:, i, c0:c0 + cn], ps[:, 0:cn],
                                BIAS[f"bv{l}"][:, c0:c0 + cn])
                        else:
                            nc.vector.tensor_copy(V_sh[:, i, c0:c0 + cn],
                                                  ps[:, 0:cn])

                v_window(0)
                v_window(1)

                # attention: unnormalized ctx + denominators; the softmax
                # divide is deferred to one batched ln/exp pass per layer
                # (keeps the ACT engine on a single table set all kernel).
                den_t = acts.tile([128, NT, FT * 128], bf,
                                  name=f"den{l}", tag="den")
                wfc = load_w(f"wfcT{l}")
                w1 = load_w(f"w1T{l}")
                xTn = new_xT(f"x1T{l}")

                def w1_chunk(c0, cn, w1=w1, xT=xTn, l=l):
                    for ft in range(FT):
                        bias = (BIAS[f"b1{l}"][:, ft:ft + 1]
                                if flags[f"b1{l}"] else 0.0)
                        ps = psg.tile([128, 512], f32, tag="gemm")
                        for kt in range(FT):
                            nc.tensor.matmul(
                                ps[:, 0:cn], w1[:, kt, ft * 128:(ft + 1) * 128],
                                xT[:, kt, 32 + c0:32 + c0 + cn],
                                start=(kt == 0), stop=(kt == FT - 1))
                        nc.scalar.activation(
                            H1T[:, ft, c0:c0 + cn], ps[:, 0:cn], AF.Relu,
                            bias=bias)

                def emit_tail(t, l=l, wfc=wfc, xTn=xTn, den_t=den_t):
                    # softmax divide rb = exp(-ln(den)), normalize ctx, then
                    # fc + residual + LN1 + transpose for tile t.
                    nc.vector.tensor_scalar_max(
                        den_t[:, t, :], den_t[:, t, :], 1e-6)
                    lnv = rpool.tile([128, T_LOC + 128], f32, tag="lnv")
                    nc.scalar.activation(lnv[:, 0:768], den_t[:, t, :], AF.Ln)
                    rb = rpool.tile([128, T_LOC + 128], bf, tag="rb")
                    nc.scalar.activation(rb[:, 0:768], lnv[:, 0:768],
                                         AF.Exp, scale=-1.0)
                    rb_r = rb[:, 0:768].rearrange("p (a b) -> p a b", b=128)
                    nc.vector.tensor_tensor(
                        out=ctxT[:, 0:FT, 128 * t:128 * t + 128],
                        in0=ctxT[:, 0:FT, 128 * t:128 * t + 128],
                        in1=rb_r, op=ALU.mult)
                    F = work.tile([128, H], f32, tag="F")
                    s1 = spool.tile([128, 2], f32, tag="s1")
                    for j, (c0, cn) in enumerate(((0, 384), (384, 384))):
                        ps = psg.tile([128, 512], f32, tag="gemm")
                        for kt in range(FT):
                            nc.tensor.matmul(
                                ps[:, 0:cn], ctxT[:, kt, 128 * t:128 * t + 128],
                                wfc[:, kt, c0:c0 + cn],
                                start=(kt == 0), stop=(kt == FT - 1))
                        nc.vector.scalar_tensor_tensor(
                            out=F[:, c0:c0 + cn], in0=ps[:, 0:cn], scalar=1.0,
                            in1=x_tok[:, t, c0:c0 + cn], op0=ALU.mult,
                            op1=ALU.add, accum_out=s1[:, j:j + 1])
                    if flags[f"bfc{l}"]:
                        nc.vector.tensor_add(F, F, BIAS[f"bfc{l}"])
                    _layernorm(nc, work, spool, F, s1, x_tok, t, eps_sb, c768,
                               BIAS.get(f"ln1g{l}"), BIAS.get(f"ln1b{l}"),
                               f32, AF, ALU)
                    transpose_set(xTn, t)

                for t in range(NT):
                    if t + 2 < FT:
                        v_window(t + 2)
                    for hp in range(FT):
                        cps = psc.tile([128, 256], f32, tag="ctx")
                        for hs in range(2):
                            h = 2 * hp + hs
                            po = 64 * hs
                            stp = pst.tile([128, 256], f32, tag="st")
                            nc.tensor.matmul(
                                stp[:, 0:128],
                                kTp[po:po + 64, hp, 128 * t:128 * t + 128],
                                qT[po:po + 64, hp, 128 * t:128 * t + 128],
                                start=True, stop=True)
                            nc.tensor.matmul(
                                stp[:, 128:256],
                                kTp[po:po + 64, hp, 128 * t + 128:128 * t + 256],
                                qT[po:po + 64, hp, 128 * t:128 * t + 128],
                                start=True, stop=True)
                            p_full = ppool.tile([128, 256], bf, tag="p_full")
                            nc.scalar.activation(p_full, stp[:, 0:256], AF.Exp)
                            nc.vector.tensor_tensor(
                                out=p_full, in0=p_full,
                                in1=maskM_sb[:, t, 0:256], op=ALU.mult)
                            nc.tensor.matmul(
                                cps[po:po + 64, 128:256], ones64[0:128, :],
                                p_full[:, 0:128], start=True, stop=False)
                            nc.tensor.matmul(
                                cps[po:po + 64, 128:256], ones64[0:64, :],
                                p_full[0:64, 128:256], start=False, stop=True)
                            nc.tensor.matmul(
                                cps[po:po + 64, 0:128],
                                V_sh[0:128, t, 64 * h:64 * h + 64],
                                p_full[:, 0:128], start=True, stop=False)
                            nc.tensor.matmul(
                                cps[po:po + 64, 0:128],
                                V_sh[0:64, t + 1, 64 * h:64 * h + 64],
                                p_full[0:64, 128:256], start=False, stop=True)
                        nc.vector.tensor_copy(
                            ctxT[:, hp, 128 * t:128 * t + 128], cps[:, 0:128])
                        nc.vector.tensor_copy(
                            den_t[:, t, 128 * hp:128 * hp + 128],
                            cps[:, 128:256])

                    # tail of the PREVIOUS tile: emitted after this
                    # tile's scores/ctx so its recip/LN chain never
                    # head-of-line blocks the PE FIFO.
                    if t == NT - 1:
                        w1_chunk(0, 384)
                    if t > 0:
                        emit_tail(t - 1)
                emit_tail(NT - 1)

                xT = xTn

                # FFN: the W1 token chunks for tokens 384+ are emitted
                # from inside the W2/LN2 tile loop (the 0:384 chunk already
                # ran inside the attention loop).
                w2 = load_w(f"w2T{l}")
                if l < L - 1:
                    xT = new_xT(f"x2T{l}")
                for t in range(NT):
                    if t < 2:
                        w1_chunk(384 + 128 * t, 128)
                    F2 = work.tile([128, H], f32, tag="F")
                    s1 = spool.tile([128, 2], f32, tag="s1")
                    for j, (c0, cn) in enumerate(((0, 384), (384, 384))):
                        ps = psg.tile([128, 512], f32, tag="gemm")
                        for kt in range(FT):
                            nc.tensor.matmul(
                                ps[:, 0:cn], H1T[:, kt, 128 * t:128 * t + 128],
                                w2[:, kt, c0:c0 + cn],
                                start=(kt == 0), stop=(kt == FT - 1))
                        nc.vector.scalar_tensor_tensor(
                            out=F2[:, c0:c0 + cn], in0=ps[:, 0:cn], scalar=1.0,
                            in1=x_tok[:, t, c0:c0 + cn], op0=ALU.mult,
                            op1=ALU.add, accum_out=s1[:, j:j + 1])
                    if flags[f"b2{l}"]:
                        nc.vector.tensor_add(F2, F2, BIAS[f"b2{l}"])
                    _layernorm(nc, work, spool, F2, s1, x_tok, t, eps_sb, c768,
                               BIAS.get(f"ln2g{l}"), BIAS.get(f"ln2b{l}"),
                               f32, AF, ALU)
                    if l < L - 1:
                        if t > 0:
                            transpose_set(xT, t - 1)
                        if t == NT - 1:
                            transpose_set(xT, t)
                    else:
                        lo = max(128 * t, HALO) - 128 * t
                        hi = min(128 * t + 128, HALO + CHUNK) - 128 * t
                        nc.sync.dma_start(
                            out=out_d[128 * t + lo - HALO:128 * t + hi - HALO, :],
                            in_=x_tok[lo:hi, t, :])

    _legalize_waits(nc)
    return nc, names


def _layernorm(nc, work, spool, F, s1, x_tok, t, eps_sb, c768,
               g_bc, b_bc, f32, AF, ALU):
    import ml_dtypes as _md  # noqa: F401
    # mean from the residual-add's accumulated row-sums; E[x^2] from an ACT
    # Square pass with accum_out (moves all stats work off the vector
    # engine). rstd = exp(-0.5*ln(var+eps)) stays on the one ACT table set.
    fsq = work.tile([128, 768], F.dtype if hasattr(F, 'dtype') else f32,
                    name="fsq", tag="fsq")
    s2 = spool.tile([128, 1], f32, tag="s2")
    nc.scalar.activation(fsq, F, AF.Square, accum_out=s2)
    mean = spool.tile([128, 1], f32, tag="mean")
    nc.vector.scalar_tensor_tensor(
        out=mean, in0=s1[:, 0:1], scalar=s1[:, 1:2], in1=c768[:, 0:1],
        op0=ALU.add, op1=ALU.mult)
    m2 = spool.tile([128, 1], f32, tag="m2")
    nc.vector.tensor_scalar_mul(m2, mean, mean[:, 0:1])
    var = spool.tile([128, 1], f32, tag="var")
    nc.vector.scalar_tensor_tensor(
        out=var, in0=s2, scalar=1.0 / 768.0, in1=m2,
        op0=ALU.mult, op1=ALU.subtract)
    lv = spool.tile([128, 1], f32, tag="lv")
    nc.scalar.activation(lv, var, AF.Ln, bias=eps_sb[:, 0:1])
    rstd = spool.tile([128, 1], f32, tag="rstd")
    nc.scalar.activation(rstd, lv, AF.Exp, scale=-0.5)
    nc.vector.tensor_scalar(
        out=x_tok[:, t, :], in0=F, scalar1=mean, scalar2=rstd,
        op0=ALU.subtract, op1=ALU.mult)
    if g_bc is not None:
        nc.vector.tensor_tensor(
            out=x_tok[:, t, :], in0=x_tok[:, t, :], in1=g_bc, op=ALU.mult)
        nc.vector.tensor_tensor(
            out=x_tok[:, t, :], in0=x_tok[:, t, :], in1=b_bc, op=ALU.add)


def run_on_device(shared, per_core, flags, trace=False):
    from concourse.bass_utils import run_bass_kernel_spmd

    nc, names = build_program(flags)
    in_maps = []
    for c in range(NCORES):
        m = {}
        for n in names:
            src = per_core[c] if n in per_core[c] else shared
            m[n] = np.ascontiguousarray(src[n])
        in_maps.append(m)
    res = run_bass_kernel_spmd(nc, in_maps, core_ids=list(range(NCORES)),
                               trace=trace)
    return [r["out"] for r in res.results], res


def kernel(**inputs):
    shared, per_core, flags = host_prep(inputs)
    core_outs, _ = run_on_device(shared, per_core, flags)
    return assemble(core_outs)


